# revision 11
# baseline (speedup 1.0000x reference)
"""Trainium2 Bass kernel for nn_Actor (blended-MoE actor network).

Computation per batch row b:
    c     = softmax(gate(x_b))                          # [4] blend coeffs
    h1    = relu(sum_e c_e (x_b @ W1_e + b1_e))         # [256]
    h2    = relu(sum_e c_e (h1  @ W2_e + b2_e))         # [128]
    mu    = sum_e c_e (h2 @ Wmu_e + bmu_e)              # [17]

Strategy (pure data-parallel over 8 NeuronCores, 16384 rows/core):
  * Feature-on-partition layout: activations are [feat, batch] tiles, so
    expert weights load directly as matmul lhsT and layer biases are
    per-partition ACT biases.  x is transposed + cast to bf16 on the host.
  * Simplex trick: sum_e c_e A_e = A_3 + sum_{e<3} c_e' (A_e - A_3) with
    c' = c[:3]; saves 1/4 of the per-expert work.
  * Scale-input blending: sum_e c_e (x @ We) = x @ W3 + sum_e ((c_e*x) @ dWe)
    so the expert blend accumulates for free in PSUM.
  * ELU via exact identity elu(z)+1 = relu(z) + min(exp(z), 1); the +1 is
    folded into the next layer's bias on the host.
  * bf16 matmuls with fp32 PSUM accumulation.
"""

import sys

for _p in ("/opt/trn_rl_repo",):
    if _p not in sys.path:
        sys.path.append(_p)

import ml_dtypes
import numpy as np

import concourse.bass as bass
import concourse.mybir as mybir
import concourse.tile as tile
from concourse import bacc
from concourse.bass_utils import run_bass_kernel_spmd

AF = mybir.ActivationFunctionType
BF16 = mybir.dt.bfloat16
F32 = mybir.dt.float32
BF = ml_dtypes.bfloat16

NCORES = 8
B_FULL = 131072
BS = B_FULL // NCORES  # 16384 rows per core
NB = 512               # batch tile (matmul free dim)
D_IN = 256
L1 = 256
L2 = 128
NA = 17
GH = 32


def build_graph(bs: int = BS, num_devices: int = NCORES):
    """Build + compile the per-core Bass graph (same graph on all cores)."""
    nc = bacc.Bacc(
        "TRN2",
        target_bir_lowering=False,
        debug=False,
        enable_asserts=False,
        num_devices=num_devices,
    )
    d = {}

    def din(name, shape, dt):
        d[name] = nc.dram_tensor(name, shape, dt, kind="ExternalInput").ap()

    din("xt", [D_IN, bs], BF16)            # x shard, transposed
    din("w1b", [2, 128, L1], BF16)         # W1[3] as [k, part, m]
    din("w1d", [3, 2, 128, L1], BF16)      # W1[e]-W1[3]
    din("w2b", [2, 128, L2], BF16)
    din("w2d", [3, 2, 128, L2], BF16)
    din("wmub", [128, NA], BF16)
    din("wmud", [3, 128, NA], BF16)
    din("gw1", [2, 128, GH], BF16)
    din("gw2", [GH, GH], BF16)
    din("gwo", [GH, 4], BF16)
    din("b1d", [3, 2, 128], BF16)          # (b1[e]-b1[3]) chunked
    din("b2d", [3, L2], BF16)
    din("bmud", [3, NA], BF16)
    din("b1b", [128, 2], F32)              # b1[3] per-partition, col per chunk
    din("b2b", [L2, 1], F32)
    din("bmub", [NA, 1], F32)
    din("gb1", [GH, 1], F32)
    din("gb2p", [GH, 1], F32)              # gb2 - colsum(gW2_bf16)
    din("gbop", [4, 1], F32)               # gbo - colsum(gWo_bf16)
    out = nc.dram_tensor("out", [NA, bs], F32, kind="ExternalOutput").ap()

    with tile.TileContext(nc) as tc:
        _body(tc, out, d, bs)
    nc.compile()
    return nc


def _body(tc, out, d, bs):
    nc = tc.nc
    nt = bs // NB

    with (
        tc.tile_pool(name="consts", bufs=1) as consts,
        tc.tile_pool(name="io", bufs=4) as io,
        tc.tile_pool(name="act", bufs=3) as act,
        tc.tile_pool(name="ps_g", bufs=2, space="PSUM") as ps_g,
        tc.tile_pool(name="ps_h1", bufs=4, space="PSUM") as ps_h1,
        tc.tile_pool(name="ps_h2", bufs=2, space="PSUM") as ps_h2,
        tc.tile_pool(name="dram", bufs=3, space="DRAM") as dram,
    ):
        # ---- load constants/weights (once) ----
        w1b_sb = consts.tile([128, 2, L1], BF16)
        w2b_sb = consts.tile([128, 2, L2], BF16)
        for k in range(2):
            nc.sync.dma_start(out=w1b_sb[:, k, :], in_=d["w1b"][k])
            nc.sync.dma_start(out=w2b_sb[:, k, :], in_=d["w2b"][k])
        w1d_sb = consts.tile([128, 3, 2, L1], BF16)
        w2d_sb = consts.tile([128, 3, 2, L2], BF16)
        for e in range(3):
            for k in range(2):
                nc.sync.dma_start(out=w1d_sb[:, e, k, :], in_=d["w1d"][e, k])
                nc.sync.dma_start(out=w2d_sb[:, e, k, :], in_=d["w2d"][e, k])
        wmub_sb = consts.tile([128, NA], BF16)
        nc.sync.dma_start(out=wmub_sb, in_=d["wmub"])
        wmud_sb = consts.tile([128, 3, NA], BF16)
        for e in range(3):
            nc.sync.dma_start(out=wmud_sb[:, e, :], in_=d["wmud"][e])
        gw1_sb = consts.tile([128, 2, GH], BF16)
        for k in range(2):
            nc.sync.dma_start(out=gw1_sb[:, k, :], in_=d["gw1"][k])
        gw2_sb = consts.tile([GH, GH], BF16)
        nc.sync.dma_start(out=gw2_sb, in_=d["gw2"])
        gwo_sb = consts.tile([GH, 4], BF16)
        nc.sync.dma_start(out=gwo_sb, in_=d["gwo"])

        b1d_sb = consts.tile([3, 2, 128], BF16)
        nc.sync.dma_start(out=b1d_sb, in_=d["b1d"])
        b2d_sb = consts.tile([3, L2], BF16)
        nc.sync.dma_start(out=b2d_sb, in_=d["b2d"])
        bmud_sb = consts.tile([3, NA], BF16)
        nc.sync.dma_start(out=bmud_sb, in_=d["bmud"])
        b1b_sb = consts.tile([128, 2], F32)
        nc.sync.dma_start(out=b1b_sb, in_=d["b1b"])
        b2b_sb = consts.tile([L2, 1], F32)
        nc.sync.dma_start(out=b2b_sb, in_=d["b2b"])
        bmub_sb = consts.tile([NA, 1], F32)
        nc.sync.dma_start(out=bmub_sb, in_=d["bmub"])
        gb1_sb = consts.tile([GH, 1], F32)
        nc.sync.dma_start(out=gb1_sb, in_=d["gb1"])
        gb2p_sb = consts.tile([GH, 1], F32)
        nc.sync.dma_start(out=gb2p_sb, in_=d["gb2p"])
        gbop_sb = consts.tile([4, 1], F32)
        nc.sync.dma_start(out=gbop_sb, in_=d["gbop"])

        ones4 = consts.tile([4, 1], F32)
        nc.vector.memset(ones4, 1.0)
        ones14 = consts.tile([1, 4], F32)
        nc.vector.memset(ones14, 1.0)

        state = {}

        def gate_phase(t):
            """Load x tile, run gate+softmax, broadcast coeffs, scale x."""
            n0 = t * NB
            xt0 = io.tile([128, NB], BF16, tag="xt0", name=f"xt0_{t}")
            nc.sync.dma_start(out=xt0, in_=d["xt"][0:128, n0 : n0 + NB])
            xt1 = io.tile([128, NB], BF16, tag="xt1", name=f"xt1_{t}")
            nc.sync.dma_start(out=xt1, in_=d["xt"][128:256, n0 : n0 + NB])

            # One PSUM bank hosts the whole gate/softmax chain:
            #   base 0:  pg1 then pden;  base 32: pg2 then pr4;  base 64: plg
            G = ps_g.tile([128, NB], F32, tag="g", name=f"G_{t}")
            pg1 = G[0:GH]
            pg2 = G[GH : 2 * GH]
            plg = G[64:68]
            pden = G[0:1]
            pr4 = G[32:36]

            def elu1(pg, gb, nm):
                """sbuf bf16 tile = elu(pg + gb) + 1 = relu(z) + min(exp(z),1)"""
                eg = act.tile([GH, NB], BF16, tag=f"eg_{nm}", name=f"eg_{nm}_{t}")
                nc.scalar.activation(eg, pg, AF.Exp, bias=gb)
                rg = act.tile([GH, NB], BF16, tag=f"rg_{nm}", name=f"rg_{nm}_{t}")
                nc.scalar.activation(rg, pg, AF.Relu, bias=gb)
                mg = act.tile([GH, NB], BF16, tag=f"mg_{nm}", name=f"mg_{nm}_{t}")
                nc.gpsimd.tensor_scalar_min(mg, eg, 1.0)
                g = act.tile([GH, NB], BF16, tag=f"g_{nm}", name=f"g_{nm}_{t}")
                nc.vector.tensor_add(g, rg, mg)
                return g

            nc.tensor.matmul(pg1, lhsT=gw1_sb[:, 0, :], rhs=xt0, start=True, stop=False)
            nc.tensor.matmul(pg1, lhsT=gw1_sb[:, 1, :], rhs=xt1, start=False, stop=True)
            g1 = elu1(pg1, gb1_sb, "1")
            nc.tensor.matmul(pg2, lhsT=gw2_sb, rhs=g1, start=True, stop=True)
            g2 = elu1(pg2, gb2p_sb, "2")
            nc.tensor.matmul(plg, lhsT=gwo_sb, rhs=g2, start=True, stop=True)
            expv = act.tile([4, NB], F32, tag="expv", name=f"expv_{t}")
            nc.scalar.activation(expv, plg, AF.Exp, bias=gbop_sb)
            nc.tensor.matmul(pden, lhsT=ones4, rhs=expv, start=True, stop=True)
            rden = act.tile([1, NB], F32, tag="rden", name=f"rden_{t}")
            nc.vector.reciprocal_approx_fast(out=rden, in_=pden)
            nc.tensor.matmul(pr4, lhsT=ones14, rhs=rden, start=True, stop=True)
            c = act.tile([4, NB], BF16, tag="c", name=f"c_{t}")
            nc.vector.tensor_mul(c, expv, pr4)

            # broadcast c rows to full-partition tiles via DRAM bounce
            c_dram = dram.tile([4, NB], BF16, tag="c_dram", name=f"c_dram_{t}")
            nc.sync.dma_start(out=c_dram, in_=c)
            cbs = []
            for e in range(3):
                cb = act.tile([128, NB], BF16, tag=f"cb{e}", name=f"cb{e}_{t}")
                nc.sync.dma_start(
                    out=cb, in_=c_dram[e : e + 1, :].to_broadcast([128, NB])
                )
                cbs.append(cb)

            xts = [xt0, xt1]
            ys1 = []
            for e in range(3):
                for k in range(2):
                    y = io.tile([128, NB], BF16, tag=f"y1_{e}_{k}", name=f"y1_{e}_{k}_{t}")
                    nc.vector.tensor_mul(y, xts[k], cbs[e])
                    ys1.append(y)
            state[t] = (xts, c, cbs, ys1)

        def heavy_phase(t):
            n0 = t * NB
            xts, c, cbs, ys1 = state.pop(t)

            # ---- layer 1 ----
            h1s = []
            ph1s = []
            for m in range(2):
                ph1 = ps_h1.tile([128, NB], F32, tag="h1", name=f"ph1_{m}_{t}")
                ph1s.append(ph1)
                ms = slice(m * 128, (m + 1) * 128)
                nc.tensor.matmul(ph1, lhsT=w1b_sb[:, 0, ms], rhs=xts[0], start=True, stop=False)
                nc.tensor.matmul(ph1, lhsT=w1b_sb[:, 1, ms], rhs=xts[1], start=False, stop=False)
                for e in range(3):
                    for k in range(2):
                        nc.tensor.matmul(
                            ph1, lhsT=w1d_sb[:, e, k, ms], rhs=ys1[e * 2 + k],
                            start=False, stop=False,
                        )
                nc.tensor.matmul(ph1, lhsT=b1d_sb[:, m, :], rhs=c[0:3, :], start=False, stop=True)
                h1 = act.tile([128, NB], BF16, tag=f"h1_{m}", name=f"h1_{m}_{t}")
                nc.scalar.activation(h1, ph1, AF.Relu, bias=b1b_sb[:, m : m + 1])
                h1s.append(h1)

            # ---- layer 2 ----
            ys2 = []
            for e in range(3):
                for k in range(2):
                    y = act.tile([128, NB], BF16, tag=f"y2_{e}_{k}", name=f"y2_{e}_{k}_{t}")
                    nc.vector.tensor_mul(y, h1s[k], cbs[e])
                    ys2.append(y)
            ph2 = ps_h2.tile([L2, NB], F32, tag="h2", name=f"ph2_{t}")
            nc.tensor.matmul(ph2, lhsT=w2b_sb[:, 0, :], rhs=h1s[0], start=True, stop=False)
            nc.tensor.matmul(ph2, lhsT=w2b_sb[:, 1, :], rhs=h1s[1], start=False, stop=False)
            for e in range(3):
                for k in range(2):
                    nc.tensor.matmul(
                        ph2, lhsT=w2d_sb[:, e, k, :], rhs=ys2[e * 2 + k],
                        start=False, stop=False,
                    )
            nc.tensor.matmul(ph2, lhsT=b2d_sb, rhs=c[0:3, :], start=False, stop=True)
            h2 = act.tile([L2, NB], BF16, tag="h2s", name=f"h2_{t}")
            nc.scalar.activation(h2, ph2, AF.Relu, bias=b2b_sb)

            # ---- output head (PSUM: reuse partitions 0:17 of ph1[m=0]) ----
            ys3 = []
            for e in range(3):
                y = act.tile([L2, NB], BF16, tag=f"y3_{e}", name=f"y3_{e}_{t}")
                nc.gpsimd.tensor_mul(y, h2, cbs[e][:L2, :])
                ys3.append(y)
            pmu = ph1s[0][0:NA]
            nc.tensor.matmul(pmu, lhsT=wmub_sb, rhs=h2, start=True, stop=False)
            for e in range(3):
                nc.tensor.matmul(pmu, lhsT=wmud_sb[:, e, :], rhs=ys3[e], start=False, stop=False)
            nc.tensor.matmul(pmu, lhsT=bmud_sb, rhs=c[0:3, :], start=False, stop=True)
            mu = act.tile([NA, NB], F32, tag="mu", name=f"mu_{t}")
            nc.scalar.activation(mu, pmu, AF.Identity, bias=bmub_sb)
            nc.sync.dma_start(out=out[:, n0 : n0 + NB], in_=mu)

        LEAD = 2
        for i in range(nt + LEAD):
            if i < nt:
                gate_phase(i)
            if i >= LEAD:
                heavy_phase(i - LEAD)


def host_prep(inputs, bs=BS, ncores=NCORES):
    """Convert full f32 inputs to per-core in_maps (weights replicated)."""
    f32 = np.float32
    x = np.asarray(inputs["x"], f32)
    W1 = np.asarray(inputs["W1"], f32)
    b1 = np.asarray(inputs["b1"], f32)
    W2 = np.asarray(inputs["W2"], f32)
    b2 = np.asarray(inputs["b2"], f32)
    Wmu = np.asarray(inputs["Wmu"], f32)
    bmu = np.asarray(inputs["bmu"], f32)
    gW1 = np.asarray(inputs["gW1"], f32)
    gb1 = np.asarray(inputs["gb1"], f32)
    gW2 = np.asarray(inputs["gW2"], f32)
    gb2 = np.asarray(inputs["gb2"], f32)
    gWo = np.asarray(inputs["gWo"], f32)
    gbo = np.asarray(inputs["gbo"], f32)

    gw2_bf = gW2.astype(BF)
    gwo_bf = gWo.astype(BF)
    common = {
        "w1b": W1[3].reshape(2, 128, L1).astype(BF),
        "w1d": (W1[:3] - W1[3]).reshape(3, 2, 128, L1).astype(BF),
        "w2b": W2[3].reshape(2, 128, L2).astype(BF),
        "w2d": (W2[:3] - W2[3]).reshape(3, 2, 128, L2).astype(BF),
        "wmub": Wmu[3].astype(BF),
        "wmud": (Wmu[:3] - Wmu[3]).astype(BF),
        "gw1": gW1.reshape(2, 128, GH).astype(BF),
        "gw2": gw2_bf,
        "gwo": gwo_bf,
        "b1d": (b1[:3] - b1[3]).reshape(3, 2, 128).astype(BF),
        "b2d": (b2[:3] - b2[3]).astype(BF),
        "bmud": (bmu[:3] - bmu[3]).astype(BF),
        "b1b": np.ascontiguousarray(b1[3].reshape(2, 128).T).astype(f32),
        "b2b": b2[3].reshape(L2, 1).astype(f32),
        "bmub": bmu[3].reshape(NA, 1).astype(f32),
        "gb1": gb1.reshape(GH, 1).astype(f32),
        "gb2p": (gb2 - gw2_bf.astype(f32).sum(0)).reshape(GH, 1).astype(f32),
        "gbop": (gbo - gwo_bf.astype(f32).sum(0)).reshape(4, 1).astype(f32),
    }
    xs = x.reshape(ncores, bs, D_IN)
    in_maps = []
    for i in range(ncores):
        m = dict(common)
        m["xt"] = xs[i].T.astype(BF)
        in_maps.append(m)
    return in_maps


_NC_CACHE = {}


def _get_nc():
    key = (BS, NCORES)
    if key not in _NC_CACHE:
        _NC_CACHE[key] = build_graph(BS, NCORES)
    return _NC_CACHE[key]


def kernel(**inputs):
    in_maps = host_prep(inputs)
    nc = _get_nc()
    res = run_bass_kernel_spmd(nc, in_maps, core_ids=list(range(NCORES)))
    outs = [m["out"] for m in res.results]  # each [17, BS] f32
    return np.concatenate([np.asarray(o, np.float32).T for o in outs], axis=0)


if __name__ == "__main__":
    # smoke build
    nc = build_graph(1024, 1)
    print("built ok")


# revision 13
# speedup vs baseline: 2.1422x; 2.1422x over previous
"""Trainium2 Bass kernel for nn_Actor (blended-MoE actor network).

Computation per batch row b:
    c     = softmax(gate(x_b))                          # [4] blend coeffs
    h1    = relu(sum_e c_e (x_b @ W1_e + b1_e))         # [256]
    h2    = relu(sum_e c_e (h1  @ W2_e + b2_e))         # [128]
    mu    = sum_e c_e (h2 @ Wmu_e + bmu_e)              # [17]

Strategy (pure data-parallel over 8 NeuronCores, 16384 rows/core):
  * Feature-on-partition layout: activations are [feat, batch] tiles, so
    expert weights load directly as matmul lhsT and layer biases are
    per-partition ACT biases.  x is transposed + cast to bf16 on the host.
  * Simplex trick: sum_e c_e A_e = A_3 + sum_{e<3} c_e' (A_e - A_3) with
    c' = c[:3]; saves 1/4 of the per-expert work.
  * Scale-input blending: sum_e c_e (x @ We) = x @ W3 + sum_e ((c_e*x) @ dWe)
    so the expert blend accumulates for free in PSUM.
  * ELU via exact identity elu(z)+1 = relu(z) + min(exp(z), 1); the +1 is
    folded into the next layer's bias on the host.
  * bf16 matmuls with fp32 PSUM accumulation.
"""

import sys

for _p in ("/opt/trn_rl_repo",):
    if _p not in sys.path:
        sys.path.append(_p)

import ml_dtypes
import numpy as np

import concourse.bass as bass
import concourse.mybir as mybir
import concourse.tile as tile
from concourse import bacc
from concourse.bass_utils import run_bass_kernel_spmd

AF = mybir.ActivationFunctionType
BF16 = mybir.dt.bfloat16
F32 = mybir.dt.float32
BF = ml_dtypes.bfloat16

NCORES = 8
B_FULL = 131072
BS = B_FULL // NCORES  # 16384 rows per core
NB = 512               # batch tile (matmul free dim)
D_IN = 256
L1 = 256
L2 = 128
NA = 17
GH = 32


def build_graph(bs: int = BS, num_devices: int = NCORES):
    """Build + compile the per-core Bass graph (same graph on all cores)."""
    nc = bacc.Bacc(
        "TRN2",
        target_bir_lowering=False,
        debug=False,
        enable_asserts=False,
        num_devices=num_devices,
    )
    d = {}

    def din(name, shape, dt):
        d[name] = nc.dram_tensor(name, shape, dt, kind="ExternalInput").ap()

    din("xt", [D_IN, bs], BF16)            # x shard, transposed
    din("w1b", [2, 128, L1], BF16)         # W1[3] as [k, part, m]
    din("w1d", [3, 2, 128, L1], BF16)      # W1[e]-W1[3]
    din("w2b", [2, 128, L2], BF16)
    din("w2d", [3, 2, 128, L2], BF16)
    din("wmub", [128, NA], BF16)
    din("wmud", [3, 128, NA], BF16)
    din("gw1", [2, 128, GH], BF16)
    din("gw2", [GH, GH], BF16)
    din("gwo", [GH, 4], BF16)
    din("b1d", [3, 2, 128], BF16)          # (b1[e]-b1[3]) chunked
    din("b2d", [3, L2], BF16)
    din("bmud", [3, NA], BF16)
    din("b1b", [128, 2], F32)              # b1[3] per-partition, col per chunk
    din("b2b", [L2, 1], F32)
    din("bmub", [NA, 1], F32)
    din("gb1", [GH, 1], F32)
    din("gb2p", [GH, 1], F32)              # gb2 - colsum(gW2_bf16)
    din("gbop", [4, 1], F32)               # gbo - colsum(gWo_bf16)
    out = nc.dram_tensor("out", [NA, bs], F32, kind="ExternalOutput").ap()

    with tile.TileContext(nc) as tc:
        _body(tc, out, d, bs)
    nc.compile()
    return nc


def _body(tc, out, d, bs):
    nc = tc.nc
    nt = bs // NB

    with (
        tc.tile_pool(name="consts", bufs=1) as consts,
        tc.tile_pool(name="io", bufs=4) as io,
        tc.tile_pool(name="act", bufs=3) as act,
        tc.tile_pool(name="ps_g", bufs=2, space="PSUM") as ps_g,
        tc.tile_pool(name="ps_h1", bufs=4, space="PSUM") as ps_h1,
        tc.tile_pool(name="ps_h2", bufs=2, space="PSUM") as ps_h2,
        tc.tile_pool(name="dram", bufs=3, space="DRAM") as dram,
    ):
        # ---- load constants/weights (once) ----
        w1b_sb = consts.tile([128, 2, L1], BF16)
        w2b_sb = consts.tile([128, 2, L2], BF16)
        for k in range(2):
            nc.sync.dma_start(out=w1b_sb[:, k, :], in_=d["w1b"][k])
            nc.sync.dma_start(out=w2b_sb[:, k, :], in_=d["w2b"][k])
        w1d_sb = consts.tile([128, 3, 2, L1], BF16)
        w2d_sb = consts.tile([128, 3, 2, L2], BF16)
        for e in range(3):
            for k in range(2):
                nc.sync.dma_start(out=w1d_sb[:, e, k, :], in_=d["w1d"][e, k])
                nc.sync.dma_start(out=w2d_sb[:, e, k, :], in_=d["w2d"][e, k])
        wmub_sb = consts.tile([128, NA], BF16)
        nc.sync.dma_start(out=wmub_sb, in_=d["wmub"])
        wmud_sb = consts.tile([128, 3, NA], BF16)
        for e in range(3):
            nc.sync.dma_start(out=wmud_sb[:, e, :], in_=d["wmud"][e])
        gw1_sb = consts.tile([128, 2, GH], BF16)
        for k in range(2):
            nc.sync.dma_start(out=gw1_sb[:, k, :], in_=d["gw1"][k])
        gw2_sb = consts.tile([GH, GH], BF16)
        nc.sync.dma_start(out=gw2_sb, in_=d["gw2"])
        gwo_sb = consts.tile([GH, 4], BF16)
        nc.sync.dma_start(out=gwo_sb, in_=d["gwo"])

        b1d_sb = consts.tile([3, 2, 128], BF16)
        nc.sync.dma_start(out=b1d_sb, in_=d["b1d"])
        b2d_sb = consts.tile([3, L2], BF16)
        nc.sync.dma_start(out=b2d_sb, in_=d["b2d"])
        bmud_sb = consts.tile([3, NA], BF16)
        nc.sync.dma_start(out=bmud_sb, in_=d["bmud"])
        b1b_sb = consts.tile([128, 2], F32)
        nc.sync.dma_start(out=b1b_sb, in_=d["b1b"])
        b2b_sb = consts.tile([L2, 1], F32)
        nc.sync.dma_start(out=b2b_sb, in_=d["b2b"])
        bmub_sb = consts.tile([NA, 1], F32)
        nc.sync.dma_start(out=bmub_sb, in_=d["bmub"])
        gb1_sb = consts.tile([GH, 1], F32)
        nc.sync.dma_start(out=gb1_sb, in_=d["gb1"])
        gb2p_sb = consts.tile([GH, 1], F32)
        nc.sync.dma_start(out=gb2p_sb, in_=d["gb2p"])
        gbop_sb = consts.tile([4, 1], F32)
        nc.sync.dma_start(out=gbop_sb, in_=d["gbop"])

        ones4 = consts.tile([4, 1], F32)
        nc.vector.memset(ones4, 1.0)
        ones14 = consts.tile([1, 4], F32)
        nc.vector.memset(ones14, 1.0)

        state = {}

        def gate_phase(t):
            """Load x tile, run gate+softmax, broadcast coeffs, scale x."""
            n0 = t * NB
            xt0 = io.tile([128, NB], BF16, tag="xt0", name=f"xt0_{t}")
            nc.sync.dma_start(out=xt0, in_=d["xt"][0:128, n0 : n0 + NB])
            xt1 = io.tile([128, NB], BF16, tag="xt1", name=f"xt1_{t}")
            nc.sync.dma_start(out=xt1, in_=d["xt"][128:256, n0 : n0 + NB])

            # One PSUM bank hosts the whole gate/softmax chain:
            #   base 0:  pg1 then pden;  base 32: pg2 then pr4;  base 64: plg
            G = ps_g.tile([128, NB], F32, tag="g", name=f"G_{t}")
            pg1 = G[0:GH]
            pg2 = G[GH : 2 * GH]
            plg = G[64:68]
            pden = G[0:1]
            pr4 = G[32:36]

            def elu1(pg, gb, nm):
                """sbuf bf16 tile = elu(pg + gb) + 1 = relu(z) + min(exp(z),1)"""
                eg = act.tile([GH, NB], BF16, tag=f"eg_{nm}", name=f"eg_{nm}_{t}")
                nc.scalar.activation(eg, pg, AF.Exp, bias=gb)
                rg = act.tile([GH, NB], BF16, tag=f"rg_{nm}", name=f"rg_{nm}_{t}")
                nc.scalar.activation(rg, pg, AF.Relu, bias=gb)
                mg = act.tile([GH, NB], BF16, tag=f"mg_{nm}", name=f"mg_{nm}_{t}")
                nc.vector.tensor_scalar_min(mg, eg, 1.0)
                g = act.tile([GH, NB], BF16, tag=f"g_{nm}", name=f"g_{nm}_{t}")
                nc.vector.tensor_add(g, rg, mg)
                return g

            nc.tensor.matmul(pg1, lhsT=gw1_sb[:, 0, :], rhs=xt0, start=True, stop=False)
            nc.tensor.matmul(pg1, lhsT=gw1_sb[:, 1, :], rhs=xt1, start=False, stop=True)
            g1 = elu1(pg1, gb1_sb, "1")
            nc.tensor.matmul(pg2, lhsT=gw2_sb, rhs=g1, start=True, stop=True)
            g2 = elu1(pg2, gb2p_sb, "2")
            nc.tensor.matmul(plg, lhsT=gwo_sb, rhs=g2, start=True, stop=True)
            expv = act.tile([4, NB], F32, tag="expv", name=f"expv_{t}")
            nc.scalar.activation(expv, plg, AF.Exp, bias=gbop_sb)
            nc.tensor.matmul(pden, lhsT=ones4, rhs=expv, start=True, stop=True)
            rden = act.tile([1, NB], F32, tag="rden", name=f"rden_{t}")
            nc.vector.reciprocal_approx_fast(out=rden, in_=pden)
            nc.tensor.matmul(pr4, lhsT=ones14, rhs=rden, start=True, stop=True)
            c = act.tile([4, NB], BF16, tag="c", name=f"c_{t}")
            nc.vector.tensor_mul(c, expv, pr4)

            # broadcast c rows to full-partition tiles via DRAM bounce
            c_dram = dram.tile([4, NB], BF16, tag="c_dram", name=f"c_dram_{t}")
            nc.sync.dma_start(out=c_dram, in_=c)
            cbs = []
            for e in range(3):
                cb = act.tile([128, NB], BF16, tag=f"cb{e}", name=f"cb{e}_{t}")
                nc.sync.dma_start(
                    out=cb, in_=c_dram[e : e + 1, :].to_broadcast([128, NB])
                )
                cbs.append(cb)

            xts = [xt0, xt1]
            ys1 = []
            for e in range(3):
                for k in range(2):
                    y = io.tile([128, NB], BF16, tag=f"y1_{e}_{k}", name=f"y1_{e}_{k}_{t}")
                    nc.vector.tensor_mul(y, xts[k], cbs[e])
                    ys1.append(y)
            state[t] = (xts, c, cbs, ys1)

        def heavy_phase(t):
            n0 = t * NB
            xts, c, cbs, ys1 = state.pop(t)

            # ---- layer 1 ----
            h1s = []
            ph1s = []
            for m in range(2):
                ph1 = ps_h1.tile([128, NB], F32, tag="h1", name=f"ph1_{m}_{t}")
                ph1s.append(ph1)
                ms = slice(m * 128, (m + 1) * 128)
                nc.tensor.matmul(ph1, lhsT=w1b_sb[:, 0, ms], rhs=xts[0], start=True, stop=False)
                nc.tensor.matmul(ph1, lhsT=w1b_sb[:, 1, ms], rhs=xts[1], start=False, stop=False)
                for e in range(3):
                    for k in range(2):
                        nc.tensor.matmul(
                            ph1, lhsT=w1d_sb[:, e, k, ms], rhs=ys1[e * 2 + k],
                            start=False, stop=False,
                        )
                nc.tensor.matmul(ph1, lhsT=b1d_sb[:, m, :], rhs=c[0:3, :], start=False, stop=True)
                h1 = act.tile([128, NB], BF16, tag=f"h1_{m}", name=f"h1_{m}_{t}")
                nc.scalar.activation(h1, ph1, AF.Relu, bias=b1b_sb[:, m : m + 1])
                h1s.append(h1)

            # ---- layer 2 ----
            ys2 = []
            for e in range(3):
                for k in range(2):
                    y = act.tile([128, NB], BF16, tag=f"y2_{e}_{k}", name=f"y2_{e}_{k}_{t}")
                    nc.vector.tensor_mul(y, h1s[k], cbs[e])
                    ys2.append(y)
            ph2 = ps_h2.tile([L2, NB], F32, tag="h2", name=f"ph2_{t}")
            nc.tensor.matmul(ph2, lhsT=w2b_sb[:, 0, :], rhs=h1s[0], start=True, stop=False)
            nc.tensor.matmul(ph2, lhsT=w2b_sb[:, 1, :], rhs=h1s[1], start=False, stop=False)
            for e in range(3):
                for k in range(2):
                    nc.tensor.matmul(
                        ph2, lhsT=w2d_sb[:, e, k, :], rhs=ys2[e * 2 + k],
                        start=False, stop=False,
                    )
            nc.tensor.matmul(ph2, lhsT=b2d_sb, rhs=c[0:3, :], start=False, stop=True)
            h2 = act.tile([L2, NB], BF16, tag="h2s", name=f"h2_{t}")
            nc.scalar.activation(h2, ph2, AF.Relu, bias=b2b_sb)

            # ---- output head (PSUM: reuse partitions 0:17 of ph1[m=0]) ----
            ys3 = []
            for e in range(3):
                y = act.tile([L2, NB], BF16, tag=f"y3_{e}", name=f"y3_{e}_{t}")
                nc.vector.tensor_mul(y, h2, cbs[e][:L2, :])
                ys3.append(y)
            pmu = ph1s[0][0:NA]
            nc.tensor.matmul(pmu, lhsT=wmub_sb, rhs=h2, start=True, stop=False)
            for e in range(3):
                nc.tensor.matmul(pmu, lhsT=wmud_sb[:, e, :], rhs=ys3[e], start=False, stop=False)
            nc.tensor.matmul(pmu, lhsT=bmud_sb, rhs=c[0:3, :], start=False, stop=True)
            mu = act.tile([NA, NB], F32, tag="mu", name=f"mu_{t}")
            nc.scalar.activation(mu, pmu, AF.Identity, bias=bmub_sb)
            nc.sync.dma_start(out=out[:, n0 : n0 + NB], in_=mu)

        LEAD = 2
        for i in range(nt + LEAD):
            if i < nt:
                gate_phase(i)
            if i >= LEAD:
                heavy_phase(i - LEAD)


def host_prep(inputs, bs=BS, ncores=NCORES):
    """Convert full f32 inputs to per-core in_maps (weights replicated)."""
    f32 = np.float32
    x = np.asarray(inputs["x"], f32)
    W1 = np.asarray(inputs["W1"], f32)
    b1 = np.asarray(inputs["b1"], f32)
    W2 = np.asarray(inputs["W2"], f32)
    b2 = np.asarray(inputs["b2"], f32)
    Wmu = np.asarray(inputs["Wmu"], f32)
    bmu = np.asarray(inputs["bmu"], f32)
    gW1 = np.asarray(inputs["gW1"], f32)
    gb1 = np.asarray(inputs["gb1"], f32)
    gW2 = np.asarray(inputs["gW2"], f32)
    gb2 = np.asarray(inputs["gb2"], f32)
    gWo = np.asarray(inputs["gWo"], f32)
    gbo = np.asarray(inputs["gbo"], f32)

    gw2_bf = gW2.astype(BF)
    gwo_bf = gWo.astype(BF)
    common = {
        "w1b": W1[3].reshape(2, 128, L1).astype(BF),
        "w1d": (W1[:3] - W1[3]).reshape(3, 2, 128, L1).astype(BF),
        "w2b": W2[3].reshape(2, 128, L2).astype(BF),
        "w2d": (W2[:3] - W2[3]).reshape(3, 2, 128, L2).astype(BF),
        "wmub": Wmu[3].astype(BF),
        "wmud": (Wmu[:3] - Wmu[3]).astype(BF),
        "gw1": gW1.reshape(2, 128, GH).astype(BF),
        "gw2": gw2_bf,
        "gwo": gwo_bf,
        "b1d": (b1[:3] - b1[3]).reshape(3, 2, 128).astype(BF),
        "b2d": (b2[:3] - b2[3]).astype(BF),
        "bmud": (bmu[:3] - bmu[3]).astype(BF),
        "b1b": np.ascontiguousarray(b1[3].reshape(2, 128).T).astype(f32),
        "b2b": b2[3].reshape(L2, 1).astype(f32),
        "bmub": bmu[3].reshape(NA, 1).astype(f32),
        "gb1": gb1.reshape(GH, 1).astype(f32),
        "gb2p": (gb2 - gw2_bf.astype(f32).sum(0)).reshape(GH, 1).astype(f32),
        "gbop": (gbo - gwo_bf.astype(f32).sum(0)).reshape(4, 1).astype(f32),
    }
    xs = x.reshape(ncores, bs, D_IN)
    in_maps = []
    for i in range(ncores):
        m = dict(common)
        m["xt"] = xs[i].T.astype(BF)
        in_maps.append(m)
    return in_maps


_NC_CACHE = {}


def _get_nc():
    key = (BS, NCORES)
    if key not in _NC_CACHE:
        _NC_CACHE[key] = build_graph(BS, NCORES)
    return _NC_CACHE[key]


def kernel(**inputs):
    in_maps = host_prep(inputs)
    nc = _get_nc()
    res = run_bass_kernel_spmd(nc, in_maps, core_ids=list(range(NCORES)))
    outs = [m["out"] for m in res.results]  # each [17, BS] f32
    return np.concatenate([np.asarray(o, np.float32).T for o in outs], axis=0)


if __name__ == "__main__":
    # smoke build
    nc = build_graph(1024, 1)
    print("built ok")
